# revision 39
# baseline (speedup 1.0000x reference)
"""Trainium2 Bass kernel for nn_DFVAE (3-stage MoE routing with sorted ids).

Strategy (hardcoded for N=16384, LD=512, experts (8, 6, 16), 8 cores):
  - Data-parallel: core c owns rows [2048c, 2048(c+1)).
  - Activations kept feature-major ([LD, rows]) and SBUF-resident across all
    three stages; z is pre-transposed on host, output transposed back on host.
  - Sorted ids => each expert owns a contiguous row segment. Every (core,
    stage) is a list of 512-row windows, each lying inside one expert piece;
    ragged tails use overlap-shifted windows (idempotent rewrites). Short
    shard-edge pieces are emitted FIRST so later in-piece windows overwrite
    the rows they wrongly touched (ACT executes writes in program order).
  - Per window: the expert's weights+bias live as one host-packed 8208B row
    per partition, fetched with four element_offset indirect-DMA gathers
    (data-driven routing, uniform SPMD program, minimal gpsimd descriptor
    generation); the row offset is loaded into PE/ACT registers and used as
    a dynamic AP offset so matmuls read the activation tile directly.
  - Matmuls in float32r (full PE rate at N>=256, ~1.5e-4 rel err per stage);
    activations stored f32r end-to-end (one rounding per stage).
    MOE_MM_DTYPE=float32 gives exact fp32 (4x slower); MOE_DYN_RHS=0 falls
    back to a DVE staging copy for the matmul rhs.
"""
import os

import numpy as np

import concourse.bass as bass
import concourse.mybir as mybir
import concourse.tile as tile
from concourse import bacc, bass_utils
from concourse.bass import ds, ts

N = 16384
LD = 512
NCORES = 8
SH = N // NCORES  # 2048 rows per core
WIN = 512
P = 128
KO = LD // P  # 4 k/m subtiles
STAGE_E = (8, 6, 16)

LAST_RESULTS = None  # test harness reads exec_time_ns off this

_program_cache = {}


def _segments(ids):
    starts = np.flatnonzero(np.diff(ids, prepend=-1))
    ends = np.append(starts[1:], len(ids))
    return list(zip(starts.tolist(), ends.tolist(), ids[starts].tolist()))


def _windows_for_core(segs, lo, hi, win):
    """(row_start, expert) windows covering [lo, hi); short edge pieces first.

    Correctness invariant (checked by caller via _legal_win): every piece
    shorter than `win` must touch a shard edge, and its wrongly-overwritten
    neighbor rows are covered by the neighbor piece's own windows, which are
    emitted later (ACT writes execute in program order)."""
    short, norm = [], []
    for a0, b0, e in segs:
        a, b = max(a0, lo), min(b0, hi)
        if a >= b:
            continue
        length = b - a
        if length < win:
            if a == lo:
                short.append((lo, e))
            elif b == hi:
                short.append((hi - win, e))
            else:
                raise AssertionError(f"interior short piece [{a},{b})")
        else:
            for i in range(length // win):
                norm.append((a + i * win, e))
            if length % win:
                norm.append((b - win, e))
    return short + norm


def _legal_win(segs, win):
    """A window size is legal if on every shard, all interior (non-shard-edge)
    pieces are >= win (edge pieces of any size are fixed up by emit order)."""
    for c in range(NCORES):
        lo, hi = c * SH, (c + 1) * SH
        for a0, b0, _ in segs:
            a, b = max(a0, lo), min(b0, hi)
            if a >= b:
                continue
            if (b - a) < win and a != lo and b != hi:
                return False
    return True


def _build_program(C, WINS, mm_dtype_name, dyn_rhs=False):
    nc = bacc.Bacc("TRN2", target_bir_lowering=False, debug=False,
                   enable_asserts=False, num_devices=NCORES)
    f32 = mybir.dt.float32
    i32 = mybir.dt.int32
    mmdt = getattr(mybir.dt, mm_dtype_name)
    ACT = mybir.EngineType.Activation
    DVE = mybir.EngineType.DVE
    PE = mybir.EngineType.PE
    Ctot = sum(C)

    # With dyn_rhs, activations live as mmdt (float32r) end-to-end: the ACT
    # writes round to f32r and matmuls read the big tile directly at a dynamic
    # offset (no DVE staging copy). Same one-rounding-per-stage numerics.
    act_dt = mmdt if dyn_rhs else f32
    WROW = KO * LD + KO  # per-partition packed row: 4x512 weights + 4 biases
    zT = nc.dram_tensor("zT_shard", [LD, SH], act_dt, kind="ExternalInput").ap()
    # host-packed per-expert rows: Wb[s][e*128+p] = [W[e][0*128+p? see host] , b]
    Wbt = [
        nc.dram_tensor(f"Wb_{s}", [STAGE_E[s] * P, WROW], f32, kind="ExternalInput").ap()
        for s in range(3)
    ]
    widx_t = nc.dram_tensor("widx", [P, Ctot], i32, kind="ExternalInput").ap()
    rowoff_t = nc.dram_tensor("rowoff", [1, Ctot], i32, kind="ExternalInput").ap()
    outT = nc.dram_tensor("outT", [LD, SH], act_dt, kind="ExternalOutput").ap()

    stage_of_slot = []
    for s in range(3):
        stage_of_slot += [s] * C[s]

    with tile.TileContext(nc) as tc:
        with (
            tc.tile_pool(name="const", bufs=1) as cpool,
            tc.tile_pool(name="w", bufs=3) as wpool,
            tc.tile_pool(name="zwin", bufs=3) as zwpool,
            tc.tile_pool(name="yt", bufs=1) as ytpool,
            tc.tile_pool(name="psum", bufs=8, space="PSUM") as ppool,
        ):
            zt_sb = cpool.tile([P, KO, SH], act_dt)
            nc.sync.dma_start(zt_sb[:], zT.rearrange("(ko p) r -> p ko r", p=P))
            desc_sb = cpool.tile([1, Ctot], i32)
            nc.sync.dma_start(desc_sb[:], rowoff_t)
            widx_sb = cpool.tile([P, Ctot], i32)
            nc.sync.dma_start(widx_sb[:], widx_t)

            # one activation buffer per stage boundary (no ping-pong reuse):
            # removes WAR edges that serialize a stage's writes against the
            # previous stage's conservatively-tracked dynamic reads
            stage_bufs = [zt_sb] + [
                ytpool.tile([P, KO, SH], act_dt, tag=f"act{i}", name=f"act{i}")
                for i in range(1, 4)
            ]
            for slot in range(Ctot):
                s = stage_of_slot[slot]
                cur, nxt = stage_bufs[s], stage_bufs[s + 1]
                # two gathers per slot over the packed 8208B rows (k01 | k23+bias):
                # halves gpsimd descriptor generation vs 4 gathers while keeping
                # matmul deps reasonably fine-grained.
                w_sb = wpool.tile([P, WROW], mmdt, tag="w")
                for g in range(KO):
                    lo = g * LD
                    hi = (g + 1) * LD + (KO if g == KO - 1 else 0)
                    nc.gpsimd.indirect_dma_start(
                        out=w_sb[:, lo:hi],
                        out_offset=None,
                        in_=Wbt[s][:],
                        in_offset=bass.IndirectOffsetOnAxis(
                            ap=widx_sb[:, slot : slot + 1], axis=0
                        ),
                        element_offset=lo,
                    )
                win = WINS[s]
                r_val = nc.values_load(
                    desc_sb[0:1, slot : slot + 1],
                    engines=[PE, ACT, DVE] if dyn_rhs else [DVE, ACT],
                    min_val=0,
                    max_val=SH - win,
                    skip_runtime_bounds_check=True,
                )
                if not dyn_rhs:
                    zwin = zwpool.tile([P, KO, win], mmdt, tag="zwin")
                    nc.vector.tensor_copy(out=zwin[:], in_=cur[:, :, ds(r_val, win)])
                chunks = [WIN] * (win // WIN) + ([win % WIN] if win % WIN else [])
                off = 0
                for sz in chunks:
                    for m in range(KO):
                        psum = ppool.tile([P, WIN], f32, tag="ps")
                        for k in range(KO):
                            nc.tensor.matmul(
                                psum[:, :sz],
                                lhsT=w_sb[:, k * LD + m * P : k * LD + (m + 1) * P],
                                rhs=cur[:, k, ds(r_val + off, sz)] if dyn_rhs
                                else zwin[:, k, off : off + sz],
                                start=(k == 0),
                                stop=(k == KO - 1),
                            )
                        bias_ap = w_sb[:, KO * LD + m : KO * LD + m + 1].bitcast(f32)
                        if dyn_rhs and m % 2 == 1:
                            # relu(psum + b) on DVE: (psum + b) max 0 —
                            # splits PSUM evacuation across ACT and DVE
                            nc.vector.tensor_scalar(
                                nxt[:, m, ds(r_val + off, sz)],
                                psum[:, :sz],
                                bias_ap,
                                0.0,
                                mybir.AluOpType.add,
                                mybir.AluOpType.max,
                            )
                        else:
                            nc.scalar.activation(
                                nxt[:, m, ds(r_val + off, sz)],
                                psum[:, :sz],
                                mybir.ActivationFunctionType.Relu,
                                bias=bias_ap,
                            )
                    off += sz
            nc.sync.dma_start(outT.rearrange("(ko p) r -> p ko r", p=P), stage_bufs[3][:])
    nc.compile()
    return nc


def _kernel_numpy_fallback(z, Ws, bs, ids_all):
    out = np.asarray(z, np.float32)
    for s in range(3):
        nxt = np.empty_like(out)
        ids = ids_all[s]
        for e in range(Ws[s].shape[0]):
            mask = ids == e
            if mask.any():
                nxt[mask] = np.maximum(out[mask] @ Ws[s][e] + bs[s][e], 0.0)
        out = nxt
    return out


def kernel(z, W_dataset, b_dataset, W_assay, b_assay, W_donor, b_donor,
           dataset_ids, assay_ids, donor_ids):
    global LAST_RESULTS
    mm_dtype_name = os.environ.get("MOE_MM_DTYPE", "float32r")

    ids_all = [
        np.asarray(dataset_ids, np.int32),
        np.asarray(assay_ids, np.int32),
        np.asarray(donor_ids, np.int32),
    ]
    Ws = [
        np.ascontiguousarray(np.asarray(W_dataset, np.float32)),
        np.ascontiguousarray(np.asarray(W_assay, np.float32)),
        np.ascontiguousarray(np.asarray(W_donor, np.float32)),
    ]
    bs = [
        np.asarray(b_dataset, np.float32),
        np.asarray(b_assay, np.float32),
        np.asarray(b_donor, np.float32),
    ]
    zT = np.ascontiguousarray(np.asarray(z, np.float32).T)  # [LD, N]

    if any(np.any(np.diff(ids) < 0) for ids in ids_all):
        return _kernel_numpy_fallback(z, Ws, bs, ids_all)
    try:
        segs_all = [_segments(ids_all[s]) for s in range(3)]

        def _pick_win(segs):
            # choose the legal window minimizing padded compute (total 512-row
            # chunk count across the padded slot grid), tie-break fewer slots
            best = None
            for w in (512, 768, 1024):
                if not _legal_win(segs, w):
                    continue
                cmax = max(
                    len(_windows_for_core(segs, c * SH, (c + 1) * SH, w))
                    for c in range(NCORES)
                )
                chunks = cmax * ((w + WIN - 1) // WIN)
                key = (chunks, cmax)
                if best is None or key < best[0]:
                    best = (key, w)
            return best[1] if best else WIN

        WINS = tuple(_pick_win(segs_all[s]) for s in range(3))
        wins = [
            [_windows_for_core(segs_all[s], c * SH, (c + 1) * SH, WINS[s])
             for c in range(NCORES)]
            for s in range(3)
        ]
    except AssertionError:
        # ids not sorted / pathological segment layout: correctness fallback
        return _kernel_numpy_fallback(z, Ws, bs, ids_all)
    C = tuple(max(len(wins[s][c]) for c in range(NCORES)) for s in range(3))
    for s in range(3):
        for c in range(NCORES):
            w = wins[s][c]
            while len(w) < C[s]:
                w.append(w[-1])
    Ctot = sum(C)

    arange = np.arange(P, dtype=np.int32)
    rowoff = np.zeros((NCORES, 1, Ctot), np.int32)
    widx = np.zeros((NCORES, P, Ctot), np.int32)
    for c in range(NCORES):
        off = 0
        for s in range(3):
            for j, (r, e) in enumerate(wins[s][c]):
                slot = off + j
                rowoff[c, 0, slot] = r - c * SH
                widx[c, :, slot] = e * P + arange
            off += C[s]

    # packed per-(expert, partition) rows: 4x512 weight cols then 4 biases
    Wb = []
    for s in range(3):
        E = STAGE_E[s]
        w_pack = Ws[s].reshape(E, KO, P, LD).transpose(0, 2, 1, 3).reshape(E, P, KO * LD)
        b_pack = bs[s].reshape(E, KO, P).transpose(0, 2, 1)  # [E, P, KO]
        Wb.append(
            np.ascontiguousarray(
                np.concatenate([w_pack, b_pack], axis=2).reshape(E * P, KO * LD + KO)
            )
        )

    dyn_rhs = os.environ.get("MOE_DYN_RHS", "1") == "1"
    key = (C, WINS, mm_dtype_name, dyn_rhs)
    if key not in _program_cache:
        _program_cache[key] = _build_program(C, WINS, mm_dtype_name, dyn_rhs)
    nc = _program_cache[key]

    in_maps = []
    for c in range(NCORES):
        m = {
            "zT_shard": np.ascontiguousarray(zT[:, c * SH : (c + 1) * SH]),
            "rowoff": rowoff[c],
            "widx": widx[c],
        }
        for s in range(3):
            m[f"Wb_{s}"] = Wb[s]
        in_maps.append(m)

    res = bass_utils.run_bass_kernel_spmd(nc, in_maps, core_ids=list(range(NCORES)))
    LAST_RESULTS = res

    out = np.empty((N, LD), np.float32)
    for c in range(NCORES):
        out[c * SH : (c + 1) * SH] = res.results[c]["outT"].T
    return out
